# revision 14
# baseline (speedup 1.0000x reference)
"""Tacotron2-style attention decoder on 8 trn2 NeuronCores.

Sharding: data-parallel over batch (B=64 -> 8 per core, params replicated).
Each core runs the full 300-step recurrence for its 8 batch rows.

Layouts (per core, B_loc=8):
  hidden states transposed: ah/dh [128, 8(chunk), 8(b)], h dim = chunk*128+p
  cell states (2*c actually): c2a/c2d [128, 64] cols (chunk, b)
  ctxT [128, 4(echunk), 8(b)]
  gates psum [128, 256] cols (m, b), m = gate*8+chunk
  attention: pm [128(a), 8(b), 480(t)], per-b s tiles [128, 480]
  aw/awc: awcat [16, 510] rows 0-7 aw, 8-15 awc; 15 zero-pad cols each side

LSTM pointwise uses sigmoid(x) = 0.5*(1+tanh(x/2)) so the steady loop only
needs the exp/tanh ACT table set (one table load total).
"""

import os
import numpy as np
import concourse.bass as bass
import concourse.tile as tile
from concourse import bacc, mybir
from concourse.bass import ts
from concourse.bass_utils import run_bass_kernel_spmd
from concourse.masks import make_identity

F32 = mybir.dt.float32
AF = mybir.ActivationFunctionType
OP = mybir.AluOpType

B, T_ENC, E = 64, 480, 512
N_MELS, NFPS, T_MEL = 80, 2, 600
ARNN, DRNN, PRE, ATTD, LOCF, LOCK = 1024, 1024, 256, 128, 32, 31
FRAME = N_MELS * NFPS          # 160
STEPS = T_MEL // NFPS          # 300
PAD = (LOCK - 1) // 2          # 15
NC = 8
BL = B // NC                   # 8 per core
TP = T_ENC + 2 * PAD           # 510 padded time axis

KA = 14   # aLSTM contraction chunks: pre(2) + ctx(4) + ah(8)
KD = 20   # dLSTM: ah(8) + ctx(4) + dh(8)
KP = 12   # proj: dh(8) + ctx(4)
M_G = 32  # gate dim 4096 / 128


def build_nc(n_steps=STEPS, unroll_python=False):
    nc = bacc.Bacc("TRN2", target_bir_lowering=False, debug=False, num_devices=NC)

    # ---- per-core inputs ----
    d_mem = nc.dram_tensor("memory_s", [BL, T_ENC, E], F32, kind="ExternalInput")
    d_din0 = nc.dram_tensor("din0", [128, STEPS, BL], F32, kind="ExternalInput")
    d_din1 = nc.dram_tensor("din1", [32, STEPS, BL], F32, kind="ExternalInput")
    # streamed big weights, rearranged per m-chunk: [M_G, K, 128, 128]
    d_wa = nc.dram_tensor("wa_r", [M_G, KA, 128, 128], F32, kind="ExternalInput")
    d_wd = nc.dram_tensor("wd_r", [M_G, KD, 128, 128], F32, kind="ExternalInput")
    # small resident weights
    d_wq = nc.dram_tensor("wq_t", [8, 128, ATTD], F32, kind="ExternalInput")
    d_wm = nc.dram_tensor("wm_t", [4, 128, ATTD], F32, kind="ExternalInput")
    d_wp = nc.dram_tensor("wp_t", [KP, 128, FRAME], F32, kind="ExternalInput")
    d_w1 = nc.dram_tensor("w1_t", [FRAME, PRE], F32, kind="ExternalInput")
    d_w2 = nc.dram_tensor("w2_t", [2, 128, PRE], F32, kind="ExternalInput")
    d_cw = nc.dram_tensor("convw", [LOCF, 2 * LOCK], F32, kind="ExternalInput")
    d_wl = nc.dram_tensor("wloc_t", [LOCF, ATTD], F32, kind="ExternalInput")
    d_v = nc.dram_tensor("v_col", [ATTD, 1], F32, kind="ExternalInput")
    d_ba = nc.dram_tensor("bias_a", [128, 256], F32, kind="ExternalInput")
    d_bd = nc.dram_tensor("bias_d", [128, 256], F32, kind="ExternalInput")
    d_pb = nc.dram_tensor("proj_b", [FRAME, 1], F32, kind="ExternalInput")

    # ---- per-core outputs (step-flattened first dim) ----
    d_spec0 = nc.dram_tensor("spec0", [STEPS * 128, BL], F32, kind="ExternalOutput")
    d_spec1 = nc.dram_tensor("spec1", [STEPS * 32, BL], F32, kind="ExternalOutput")
    d_align = nc.dram_tensor("align", [STEPS * BL, T_ENC], F32, kind="ExternalOutput")

    with tile.TileContext(nc) as tc:
        with (
            tc.tile_pool(name="const", bufs=1) as cpool,
            tc.tile_pool(name="state", bufs=1) as spool,
        ):
            # ---------- resident tiles ----------
            wq = cpool.tile([128, 8, ATTD], F32)
            nc.sync.dma_start(out=wq, in_=d_wq.ap().rearrange("k p a -> p k a"))
            wm = cpool.tile([128, 4, ATTD], F32)
            nc.sync.dma_start(out=wm, in_=d_wm.ap().rearrange("k p a -> p k a"))
            wp = cpool.tile([128, KP, FRAME], F32)
            nc.sync.dma_start(out=wp, in_=d_wp.ap().rearrange("k p a -> p k a"))
            w1a = cpool.tile([128, PRE], F32)
            nc.sync.dma_start(out=w1a, in_=d_w1.ap()[0:128, :])
            w1b = cpool.tile([32, PRE], F32)
            nc.sync.dma_start(out=w1b, in_=d_w1.ap()[128:160, :])
            w2 = cpool.tile([128, 2, PRE], F32)
            nc.sync.dma_start(out=w2, in_=d_w2.ap().rearrange("k p a -> p k a"))
            cw = cpool.tile([LOCF, 2 * LOCK], F32)
            nc.sync.dma_start(out=cw, in_=d_cw.ap())
            wl = cpool.tile([LOCF, ATTD], F32)
            nc.sync.dma_start(out=wl, in_=d_wl.ap())
            v_col = cpool.tile([ATTD, 1], F32)
            nc.sync.dma_start(out=v_col, in_=d_v.ap())
            bias_a = cpool.tile([128, 256], F32)
            nc.sync.dma_start(out=bias_a, in_=d_ba.ap())
            bias_d = cpool.tile([128, 256], F32)
            nc.sync.dma_start(out=bias_d, in_=d_bd.ap())
            pb0 = cpool.tile([128, 1], F32)
            nc.sync.dma_start(out=pb0, in_=d_pb.ap()[0:128, :])
            pb1 = cpool.tile([32, 1], F32)
            nc.sync.dma_start(out=pb1, in_=d_pb.ap()[128:160, :])

            # memory in [t-part, b, tchunk, e] layout (ctx moving operand)
            # t axis chunked as 4 x 120 (480 = 4*120)
            mem_sb = cpool.tile([120, BL, 4, E], F32)
            nc.sync.dma_start(
                out=mem_sb,
                in_=d_mem.ap().rearrange("b (tc p) e -> p b tc e", p=120),
            )

            ident = cpool.tile([128, 128], F32)
            make_identity(nc, ident)
            pre2 = cpool.tile([128, 2, STEPS * BL], F32)       # prenet output.T
            pm = cpool.tile([128, BL, T_ENC], F32)             # processed memory.T
            w2t = cpool.tile([2 * LOCK, ATTD], F32)            # fused conv+loc-proj

            # ---------- state tiles ----------
            ah = spool.tile([128, 8, BL], F32)
            c2a = spool.tile([128, 8 * BL], F32)
            dh = spool.tile([128, 8, BL], F32)
            c2d = spool.tile([128, 8 * BL], F32)
            ctxT = spool.tile([128, 4, BL], F32)
            aw_t = spool.tile([BL, TP], F32)
            awc_t = spool.tile([BL, TP], F32)
            for t_ in (ah, c2a, dh, c2d, ctxT, aw_t, awc_t):
                nc.vector.memset(t_, 0.0)

            # ---------- P0: one-time precompute ----------
            with (
                tc.tile_pool(name="p0tmp", bufs=2) as tpool,
                tc.tile_pool(name="p0psum", bufs=2, space="PSUM") as pps,
                tc.tile_pool(name="p0big", bufs=1) as bigpool,
            ):
                din0 = bigpool.tile([128, STEPS, BL], F32)
                nc.sync.dma_start(out=din0, in_=d_din0.ap())
                din1 = bigpool.tile([32, STEPS, BL], F32)
                nc.sync.dma_start(out=din1, in_=d_din1.ap())

                ps_w2 = pps.tile([2 * LOCK, ATTD], F32, tag="pp")
                nc.tensor.matmul(ps_w2, cw, wl, start=True, stop=True)
                nc.vector.tensor_copy(w2t, ps_w2)

                # prenet layer 1: pre1.T [256, 2400] = relu(w1T.T @ din)
                NSTEP = STEPS * BL  # 2400
                NCH = NSTEP // 480  # 5 chunks of 480
                din0f = din0.rearrange("p t b -> p (t b)")
                din1f = din1.rearrange("p t b -> p (t b)")
                pre1 = bigpool.tile([128, 2, NSTEP], F32)
                for m in range(2):
                    for nch in range(NCH):
                        sl = ts(nch, 480)
                        p = pps.tile([128, 480], F32, tag="pp")
                        nc.tensor.matmul(p, w1a[:, ts(m, 128)], din0f[:, sl],
                                         start=True, stop=False)
                        nc.tensor.matmul(p, w1b[:, ts(m, 128)], din1f[:, sl],
                                         start=False, stop=True)
                        nc.scalar.activation(pre1[:, m, sl], p, AF.Relu)
                for m in range(2):
                    for nch in range(NCH):
                        sl = ts(nch, 480)
                        p = pps.tile([128, 480], F32, tag="pp")
                        for k in range(2):
                            nc.tensor.matmul(p, w2[:, k, ts(m, 128)], pre1[:, k, sl],
                                             start=(k == 0), stop=(k == 1))
                        nc.scalar.activation(pre2[:, m, sl], p, AF.Relu)

                # mem transposed [e%128, ec, b, tc, t%128] then pm = wmT.T @ memT
                for b in range(BL):
                    memT_b = tpool.tile([128, 4, 4, 120], F32, tag="memT")
                    for tc_i in range(4):
                        for ec in range(4):
                            pt = pps.tile([128, 120], F32, tag="pp")
                            nc.tensor.transpose(
                                pt, mem_sb[:, b, tc_i, ts(ec, 128)],
                                ident[0:120, 0:120])
                            nc.vector.tensor_copy(memT_b[:, ec, tc_i, :], pt)
                    pmp = pps.tile([128, T_ENC], F32, tag="pp")
                    for ec in range(4):
                        nc.tensor.matmul(
                            pmp, wm[:, ec, :],
                            memT_b[:, ec, :, :].rearrange("p tc t -> p (tc t)"),
                            start=(ec == 0), stop=(ec == 3))
                    nc.vector.tensor_copy(pm[:, b, :], pmp)

            # ---------- steady-state loop ----------
            with (
                tc.tile_pool(name="wbuf", bufs=2) as wpool,
                tc.tile_pool(name="step", bufs=2) as stp,
                tc.tile_pool(name="im2col", bufs=1) as imp,
                tc.tile_pool(name="srow", bufs=2) as srp,
                tc.tile_pool(name="pstep", bufs=2, space="PSUM") as psp,
            ):
                def lstm_pointwise(gates, c2x, h_out):
                    """gates [128,256] preact (i,f,g,o)x(chunk,b); updates c2x (=2c), h_out."""
                    gi, gf = gates[:, 0:64], gates[:, 64:128]
                    gg, go = gates[:, 128:192], gates[:, 192:256]
                    ti = stp.tile([128, 64], F32, tag="ti")
                    tf = stp.tile([128, 64], F32, tag="tf")
                    tg = stp.tile([128, 64], F32, tag="tg")
                    to = stp.tile([128, 64], F32, tag="to")
                    nc.scalar.activation(ti, gi, AF.Tanh, scale=0.5)
                    nc.scalar.activation(tf, gf, AF.Tanh, scale=0.5)
                    nc.scalar.activation(tg, gg, AF.Tanh)
                    nc.scalar.activation(to, go, AF.Tanh, scale=0.5)
                    m1 = stp.tile([128, 64], F32, tag="m1")
                    nc.vector.tensor_tensor(m1, tf, c2x, OP.mult)      # tf*2c
                    a1 = stp.tile([128, 64], F32, tag="a1")
                    nc.vector.tensor_tensor(a1, m1, c2x, OP.add)       # (1+tf)*2c
                    m2 = stp.tile([128, 64], F32, tag="m2")
                    nc.vector.tensor_tensor(m2, ti, tg, OP.mult)
                    a2 = stp.tile([128, 64], F32, tag="a2")
                    nc.vector.tensor_tensor(a2, m2, tg, OP.add)        # (1+ti)*tg
                    # c2x_new = 2*(sig(f)c + sig(i)tg) = 0.5*a1 + a2
                    nc.vector.scalar_tensor_tensor(
                        out=c2x, in0=a1, scalar=0.5, in1=a2,
                        op0=OP.mult, op1=OP.add)
                    thc = stp.tile([128, 64], F32, tag="thc")
                    nc.scalar.activation(thc, c2x, AF.Tanh, scale=0.5)  # tanh(c)
                    h1 = stp.tile([128, 64], F32, tag="h1")
                    nc.vector.tensor_tensor(h1, to, thc, OP.mult)
                    nc.vector.tensor_tensor(h1, h1, thc, OP.add)
                    nc.vector.tensor_scalar_mul(h_out, h1, 0.5)

                def body(i):
                    # ---- aLSTM gates ----
                    pg = psp.tile([128, 256], F32, tag="gates")
                    for m in range(M_G):
                        wa_m = wpool.tile([128, KA, 128], F32, tag="wa")
                        nc.sync.dma_start(
                            out=wa_m, in_=d_wa.ap()[m].rearrange("k p q -> p k q"))
                        for k in range(KA):
                            if k < 2:
                                rhs = pre2[:, k, ts(i, BL)]
                            elif k < 6:
                                rhs = ctxT[:, k - 2, :]
                            else:
                                rhs = ah[:, k - 6, :]
                            nc.tensor.matmul(
                                pg[:, ts(m, BL)], wa_m[:, k, :], rhs,
                                start=(k == 0), stop=(k == KA - 1))
                    gsb = stp.tile([128, 256], F32, tag="gsb")
                    nc.vector.tensor_tensor(gsb, pg, bias_a, OP.add)
                    lstm_pointwise(gsb, c2a, ah.rearrange("p c b -> p (c b)"))

                    # ---- attention ----
                    pq = psp.tile([128, BL], F32, tag="loc")
                    for k in range(8):
                        nc.tensor.matmul(pq, wq[:, k, :], ah[:, k, :],
                                         start=(k == 0), stop=(k == 7))
                    qT = stp.tile([128, BL], F32, tag="qT")
                    nc.vector.tensor_copy(qT, pq)

                    # im2col: rhs62 [62, b, t], row k reads aw[b, k:k+480]
                    rhs62 = imp.tile([2 * LOCK, BL, T_ENC], F32, tag="rhs62")
                    for kk in range(LOCK):
                        nc.sync.dma_start(
                            out=rhs62[kk:kk + 1, :, :],
                            in_=aw_t[:, kk:kk + T_ENC])
                        nc.sync.dma_start(
                            out=rhs62[LOCK + kk:LOCK + kk + 1, :, :],
                            in_=awc_t[:, kk:kk + T_ENC])

                    e_sb = stp.tile([BL, T_ENC], F32, tag="e_sb")
                    for b in range(BL):
                        ploc_b = psp.tile([128, T_ENC], F32, tag="loc")
                        nc.tensor.matmul(ploc_b, w2t, rhs62[:, b, :],
                                         start=True, stop=True)
                        s1 = srp.tile([128, T_ENC], F32, tag="s1")
                        nc.vector.tensor_tensor(s1, ploc_b, pm[:, b, :], OP.add)
                        s2 = srp.tile([128, T_ENC], F32, tag="s2")
                        nc.scalar.activation(s2, s1, AF.Tanh, bias=qT[:, b:b + 1])
                        e_b = psp.tile([1, 512], F32, tag="row")
                        nc.tensor.matmul(e_b[0:1, 0:T_ENC], v_col, s2,
                                         start=True, stop=True)
                        erow = srp.tile([1, T_ENC], F32, tag="erow")
                        nc.scalar.copy(erow, e_b[0:1, 0:T_ENC])
                        nc.sync.dma_start(out=e_sb[b:b + 1, :], in_=erow)

                    # softmax over t (|e| <= sum|v| so exp is safe without max-sub)
                    esum = stp.tile([BL, 1], F32, tag="esum")
                    expe = stp.tile([BL, T_ENC], F32, tag="expe")
                    nc.scalar.activation(expe, e_sb, AF.Exp, accum_out=esum)
                    rsum = stp.tile([BL, 1], F32, tag="rsum")
                    nc.vector.reciprocal(rsum, esum)
                    nc.vector.tensor_scalar(
                        out=aw_t[:, PAD:PAD + T_ENC], in0=expe,
                        scalar1=rsum, scalar2=None, op0=OP.mult)
                    nc.vector.tensor_tensor(
                        awc_t[:, PAD:PAD + T_ENC],
                        awc_t[:, PAD:PAD + T_ENC],
                        aw_t[:, PAD:PAD + T_ENC], OP.add)
                    nc.sync.dma_start(
                        out=d_align.ap()[ts(i, BL)],
                        in_=aw_t[:, PAD:PAD + T_ENC])

                    # w transposed [t%128, tc, b] then ctx rows -> ctxT scatter
                    w_t = stp.tile([120, 4, BL], F32, tag="w_t")
                    for tc_i in range(4):
                        ptw = psp.tile([120, BL], F32, tag="loc")
                        nc.tensor.transpose(
                            ptw, aw_t[:, PAD + 120 * tc_i:PAD + 120 * (tc_i + 1)],
                            ident[0:BL, 0:BL])
                        nc.vector.tensor_copy(w_t[:, tc_i, :], ptw)
                    ctx_rows = stp.tile([BL, E], F32, tag="ctx_rows")
                    for b in range(BL):
                        ctx_b = psp.tile([1, 512], F32, tag="row")
                        for tc_i in range(4):
                            nc.tensor.matmul(
                                ctx_b, w_t[:, tc_i, b:b + 1],
                                mem_sb[:, b, tc_i, :],
                                start=(tc_i == 0), stop=(tc_i == 3))
                        crow = srp.tile([1, E], F32, tag="crow")
                        nc.scalar.copy(crow, ctx_b)
                        nc.sync.dma_start(out=ctx_rows[b:b + 1, :], in_=crow)
                    for ec in range(4):
                        ptc = psp.tile([128, BL], F32, tag="loc")
                        nc.tensor.transpose(
                            ptc, ctx_rows[:, ts(ec, 128)], ident[0:BL, 0:BL])
                        nc.vector.tensor_copy(ctxT[:, ec, :], ptc)

                    # ---- dLSTM ----
                    pgd = psp.tile([128, 256], F32, tag="gates")
                    for m in range(M_G):
                        wd_m = wpool.tile([128, KD, 128], F32, tag="wd")
                        nc.sync.dma_start(
                            out=wd_m, in_=d_wd.ap()[m].rearrange("k p q -> p k q"))
                        for k in range(KD):
                            if k < 8:
                                rhs = ah[:, k, :]
                            elif k < 12:
                                rhs = ctxT[:, k - 8, :]
                            else:
                                rhs = dh[:, k - 12, :]
                            nc.tensor.matmul(
                                pgd[:, ts(m, BL)], wd_m[:, k, :], rhs,
                                start=(k == 0), stop=(k == KD - 1))
                    gsd = stp.tile([128, 256], F32, tag="gsb")
                    nc.vector.tensor_tensor(gsd, pgd, bias_d, OP.add)
                    lstm_pointwise(gsd, c2d, dh.rearrange("p c b -> p (c b)"))

                    # ---- projection ----
                    po0 = psp.tile([128, BL], F32, tag="loc")
                    for k in range(KP):
                        rhs = dh[:, k, :] if k < 8 else ctxT[:, k - 8, :]
                        nc.tensor.matmul(po0, wp[:, k, 0:128], rhs,
                                         start=(k == 0), stop=(k == KP - 1))
                    so0 = stp.tile([128, BL], F32, tag="so0")
                    nc.vector.tensor_scalar(out=so0, in0=po0, scalar1=pb0,
                                            scalar2=None, op0=OP.add)
                    nc.sync.dma_start(out=d_spec0.ap()[ts(i, 128)], in_=so0)
                    po1 = psp.tile([32, BL], F32, tag="loc")
                    for k in range(KP):
                        rhs = dh[:, k, :] if k < 8 else ctxT[:, k - 8, :]
                        nc.tensor.matmul(po1, wp[:, k, 128:160], rhs,
                                         start=(k == 0), stop=(k == KP - 1))
                    so1 = stp.tile([32, BL], F32, tag="so1")
                    nc.vector.tensor_scalar(out=so1, in0=po1, scalar1=pb1,
                                            scalar2=None, op0=OP.add)
                    nc.sync.dma_start(out=d_spec1.ap()[ts(i, 32)], in_=so1)

                if unroll_python:
                    for i in range(n_steps):
                        body(i)
                else:
                    with tc.For_i(0, n_steps) as i:
                        body(i)

    nc.finalize()
    return nc


def _host_prep(inputs):
    """Build per-core in_maps from full inputs (numpy reshapes only)."""
    mem = np.ascontiguousarray(inputs["memory"], np.float32)
    target = np.ascontiguousarray(inputs["target"], np.float32)

    x = target.transpose(0, 2, 1).reshape(B, STEPS, FRAME)
    inp = np.concatenate([np.zeros((B, 1, FRAME), np.float32), x[:, :STEPS - 1]], 1)
    din = np.ascontiguousarray(inp.transpose(2, 1, 0))  # [160, 300, B]

    wih_a = inputs["arnn_wih"].astype(np.float32)
    whh_a = inputs["arnn_whh"].astype(np.float32)
    wih_d = inputs["drnn_wih"].astype(np.float32)
    whh_d = inputs["drnn_whh"].astype(np.float32)

    wa_t = np.concatenate([wih_a.T, whh_a.T], 0)    # [1792, 4096] rows: pre,ctx,ah
    wd_t = np.concatenate([wih_d.T, whh_d.T], 0)    # [2560, 4096] rows: ah,ctx,dh
    wa_r = np.ascontiguousarray(
        wa_t.reshape(KA, 128, M_G, 128).transpose(2, 0, 1, 3))
    wd_r = np.ascontiguousarray(
        wd_t.reshape(KD, 128, M_G, 128).transpose(2, 0, 1, 3))

    wq_t = np.ascontiguousarray(inputs["att_wq"].T.reshape(8, 128, ATTD), np.float32)
    wm_t = np.ascontiguousarray(inputs["att_wmem"].T.reshape(4, 128, ATTD), np.float32)
    wp_t = np.ascontiguousarray(inputs["proj_w"].T.reshape(KP, 128, FRAME), np.float32)
    w1_t = np.ascontiguousarray(inputs["prenet_w1"].T, np.float32)
    w2_t = np.ascontiguousarray(inputs["prenet_w2"].T.reshape(2, 128, PRE), np.float32)
    convw = np.ascontiguousarray(inputs["att_loc_conv"].reshape(LOCF, 2 * LOCK), np.float32)
    wl_t = np.ascontiguousarray(inputs["att_wloc"].T, np.float32)
    v_col = np.ascontiguousarray(inputs["att_v"].T, np.float32)

    ba = (inputs["arnn_bih"] + inputs["arnn_bhh"]).astype(np.float32).reshape(M_G, 128)
    bias_a = np.ascontiguousarray(np.repeat(ba.T[:, :, None], BL, 2).reshape(128, 256))
    bd = (inputs["drnn_bih"] + inputs["drnn_bhh"]).astype(np.float32).reshape(M_G, 128)
    bias_d = np.ascontiguousarray(np.repeat(bd.T[:, :, None], BL, 2).reshape(128, 256))
    pb = np.ascontiguousarray(inputs["proj_b"].reshape(FRAME, 1), np.float32)

    shared = dict(
        wa_r=wa_r, wd_r=wd_r, wq_t=wq_t, wm_t=wm_t, wp_t=wp_t,
        w1_t=w1_t, w2_t=w2_t, convw=convw, wloc_t=wl_t, v_col=v_col,
        bias_a=bias_a, bias_d=bias_d, proj_b=pb,
    )
    in_maps = []
    for c in range(NC):
        bs = slice(c * BL, (c + 1) * BL)
        m = dict(shared)
        m["memory_s"] = np.ascontiguousarray(mem[bs])
        m["din0"] = np.ascontiguousarray(din[0:128, :, bs])
        m["din1"] = np.ascontiguousarray(din[128:160, :, bs])
        in_maps.append(m)
    return in_maps


def _host_post(results, n_steps=STEPS):
    specs = np.zeros((STEPS, B, FRAME), np.float32)
    aligns = np.zeros((B, STEPS, T_ENC), np.float32)
    for c, r in enumerate(results):
        bs = slice(c * BL, (c + 1) * BL)
        s0 = r["spec0"].reshape(STEPS, 128, BL)
        s1 = r["spec1"].reshape(STEPS, 32, BL)
        specs[:, bs, 0:128] = s0.transpose(0, 2, 1)
        specs[:, bs, 128:160] = s1.transpose(0, 2, 1)
        aligns[bs] = r["align"].reshape(STEPS, BL, T_ENC).transpose(1, 0, 2)
    spec_out = specs.transpose(1, 0, 2).reshape(B, STEPS * NFPS, N_MELS).transpose(0, 2, 1)
    return spec_out, aligns


_BUILT = {}


def kernel(**inputs):
    n_steps = int(os.environ.get("KSTEPS", STEPS))
    unroll = os.environ.get("KUNROLL", "0") == "1"
    key = (n_steps, unroll)
    if key not in _BUILT:
        _BUILT[key] = build_nc(n_steps, unroll)
    nc = _BUILT[key]
    in_maps = _host_prep(inputs)
    trace = os.environ.get("KTRACE", "0") == "1"
    res = run_bass_kernel_spmd(nc, in_maps, core_ids=list(range(NC)), trace=trace)
    kernel.last_results = res
    return _host_post(res.results, n_steps)


# revision 16
# speedup vs baseline: 1.2651x; 1.2651x over previous
"""Tacotron2-style attention decoder on 8 trn2 NeuronCores.

Sharding: data-parallel over batch (B=64 -> 8 per core, params replicated).
Each core runs the full 300-step recurrence for its 8 batch rows.

Layouts (per core, B_loc=8):
  hidden states transposed: ah/dh [128, 8(chunk), 8(b)], h dim = chunk*128+p
  cell states (2*c actually): c2a/c2d [128, 64] cols (chunk, b)
  ctxT [128, 4(echunk), 8(b)]
  gates psum [128, 256] cols (m, b), m = gate*8+chunk
  attention: pm [128(a), 8(b), 480(t)], per-b s tiles [128, 480]
  aw/awc: awcat [16, 510] rows 0-7 aw, 8-15 awc; 15 zero-pad cols each side

LSTM pointwise uses sigmoid(x) = 0.5*(1+tanh(x/2)) so the steady loop only
needs the exp/tanh ACT table set (one table load total).
"""

import os
import ml_dtypes
import numpy as np
import concourse.bass as bass
import concourse.tile as tile
from concourse import bacc, mybir
from concourse.bass import ts
from concourse.bass_utils import run_bass_kernel_spmd
from concourse.masks import make_identity

F32 = mybir.dt.float32
BF16 = mybir.dt.bfloat16
FP8 = mybir.dt.float8e4
AF = mybir.ActivationFunctionType
OP = mybir.AluOpType

B, T_ENC, E = 64, 480, 512
N_MELS, NFPS, T_MEL = 80, 2, 600
ARNN, DRNN, PRE, ATTD, LOCF, LOCK = 1024, 1024, 256, 128, 32, 31
FRAME = N_MELS * NFPS          # 160
STEPS = T_MEL // NFPS          # 300
PAD = (LOCK - 1) // 2          # 15
NC = 8
BL = B // NC                   # 8 per core
TP = T_ENC + 2 * PAD           # 510 padded time axis

KA = 14   # aLSTM contraction chunks: pre(2) + ctx(4) + ah(8)
KD = 20   # dLSTM: ah(8) + ctx(4) + dh(8)
KP = 12   # proj: dh(8) + ctx(4)
M_G = 32  # gate dim 4096 / 128


def build_nc(n_steps=STEPS, unroll_python=False):
    nc = bacc.Bacc("TRN2", target_bir_lowering=False, debug=False, num_devices=NC)

    # ---- per-core inputs ----
    d_mem = nc.dram_tensor("memory_s", [BL, T_ENC, E], F32, kind="ExternalInput")
    d_din0 = nc.dram_tensor("din0", [128, STEPS, BL], F32, kind="ExternalInput")
    d_din1 = nc.dram_tensor("din1", [32, STEPS, BL], F32, kind="ExternalInput")
    # streamed big weights, rearranged per m-chunk: [M_G, K, 128, 128]
    d_wa = nc.dram_tensor("wa_r", [M_G, KA, 128, 128], FP8, kind="ExternalInput")
    d_wd = nc.dram_tensor("wd_r", [M_G, KD, 128, 128], FP8, kind="ExternalInput")
    # small resident weights
    d_wq = nc.dram_tensor("wq_t", [8, 128, ATTD], BF16, kind="ExternalInput")
    d_wm = nc.dram_tensor("wm_t", [4, 128, ATTD], F32, kind="ExternalInput")
    d_wp = nc.dram_tensor("wp_t", [KP, 128, FRAME], BF16, kind="ExternalInput")
    d_w1 = nc.dram_tensor("w1_t", [FRAME, PRE], F32, kind="ExternalInput")
    d_w2 = nc.dram_tensor("w2_t", [2, 128, PRE], F32, kind="ExternalInput")
    d_cw = nc.dram_tensor("convw", [LOCF, 2 * LOCK], F32, kind="ExternalInput")
    d_wl = nc.dram_tensor("wloc_t", [LOCF, ATTD], F32, kind="ExternalInput")
    d_v = nc.dram_tensor("v_col", [ATTD, 1], F32, kind="ExternalInput")
    d_ba = nc.dram_tensor("bias_a", [128, 256], F32, kind="ExternalInput")
    d_bd = nc.dram_tensor("bias_d", [128, 256], F32, kind="ExternalInput")
    d_pb = nc.dram_tensor("proj_b", [FRAME, 1], F32, kind="ExternalInput")

    # ---- per-core outputs (step-flattened first dim) ----
    d_spec0 = nc.dram_tensor("spec0", [STEPS * 128, BL], F32, kind="ExternalOutput")
    d_spec1 = nc.dram_tensor("spec1", [STEPS * 32, BL], F32, kind="ExternalOutput")
    d_align = nc.dram_tensor("align", [STEPS * BL, T_ENC], F32, kind="ExternalOutput")

    with tile.TileContext(nc) as tc:
        with (
            tc.tile_pool(name="const", bufs=1) as cpool,
            tc.tile_pool(name="state", bufs=1) as spool,
        ):
            # ---------- resident tiles ----------
            wq = cpool.tile([128, 8, ATTD], BF16)
            nc.sync.dma_start(out=wq, in_=d_wq.ap().rearrange("k p a -> p k a"))
            wm = cpool.tile([128, 4, ATTD], F32)
            nc.sync.dma_start(out=wm, in_=d_wm.ap().rearrange("k p a -> p k a"))
            wp = cpool.tile([128, KP, FRAME], BF16)
            nc.sync.dma_start(out=wp, in_=d_wp.ap().rearrange("k p a -> p k a"))
            w1a = cpool.tile([128, PRE], F32)
            nc.sync.dma_start(out=w1a, in_=d_w1.ap()[0:128, :])
            w1b = cpool.tile([32, PRE], F32)
            nc.sync.dma_start(out=w1b, in_=d_w1.ap()[128:160, :])
            w2 = cpool.tile([128, 2, PRE], F32)
            nc.sync.dma_start(out=w2, in_=d_w2.ap().rearrange("k p a -> p k a"))
            cw = cpool.tile([LOCF, 2 * LOCK], F32)
            nc.sync.dma_start(out=cw, in_=d_cw.ap())
            wl = cpool.tile([LOCF, ATTD], F32)
            nc.sync.dma_start(out=wl, in_=d_wl.ap())
            v_col = cpool.tile([ATTD, 1], F32)
            nc.sync.dma_start(out=v_col, in_=d_v.ap())
            bias_a = cpool.tile([128, 256], F32)
            nc.sync.dma_start(out=bias_a, in_=d_ba.ap())
            bias_d = cpool.tile([128, 256], F32)
            nc.sync.dma_start(out=bias_d, in_=d_bd.ap())
            pb0 = cpool.tile([128, 1], F32)
            nc.sync.dma_start(out=pb0, in_=d_pb.ap()[0:128, :])
            pb1 = cpool.tile([32, 1], F32)
            nc.sync.dma_start(out=pb1, in_=d_pb.ap()[128:160, :])

            # memory in [t-part, b, tchunk, e] layout (ctx moving operand)
            # t axis chunked as 4 x 120 (480 = 4*120)
            mem_sb = cpool.tile([120, BL, 4, E], F32)
            nc.sync.dma_start(
                out=mem_sb,
                in_=d_mem.ap().rearrange("b (tc p) e -> p b tc e", p=120),
            )

            ident = cpool.tile([128, 128], F32)
            make_identity(nc, ident)
            pre2 = cpool.tile([128, 2, STEPS * BL], BF16)      # prenet output.T
            pm = cpool.tile([128, BL, T_ENC], F32)             # processed memory.T
            w2t = cpool.tile([2 * LOCK, ATTD], F32)            # fused conv+loc-proj

            # ---------- state tiles ----------
            ah = spool.tile([128, 8, BL], BF16)
            c2a = spool.tile([128, 8 * BL], F32)
            dh = spool.tile([128, 8, BL], BF16)
            c2d = spool.tile([128, 8 * BL], F32)
            ctxT = spool.tile([128, 4, BL], BF16)
            aw_t = spool.tile([BL, TP], F32)
            awc_t = spool.tile([BL, TP], F32)
            for t_ in (ah, c2a, dh, c2d, ctxT, aw_t, awc_t):
                nc.vector.memset(t_, 0.0)

            # ---------- P0: one-time precompute ----------
            with (
                tc.tile_pool(name="p0tmp", bufs=2) as tpool,
                tc.tile_pool(name="p0psum", bufs=2, space="PSUM") as pps,
                tc.tile_pool(name="p0big", bufs=1) as bigpool,
            ):
                din0 = bigpool.tile([128, STEPS, BL], F32)
                nc.sync.dma_start(out=din0, in_=d_din0.ap())
                din1 = bigpool.tile([32, STEPS, BL], F32)
                nc.sync.dma_start(out=din1, in_=d_din1.ap())

                ps_w2 = pps.tile([2 * LOCK, ATTD], F32, tag="pp")
                nc.tensor.matmul(ps_w2, cw, wl, start=True, stop=True)
                nc.vector.tensor_copy(w2t, ps_w2)

                # prenet layer 1: pre1.T [256, 2400] = relu(w1T.T @ din)
                NSTEP = STEPS * BL  # 2400
                NCH = NSTEP // 480  # 5 chunks of 480
                din0f = din0.rearrange("p t b -> p (t b)")
                din1f = din1.rearrange("p t b -> p (t b)")
                pre1 = bigpool.tile([128, 2, NSTEP], F32)
                for m in range(2):
                    for nch in range(NCH):
                        sl = ts(nch, 480)
                        p = pps.tile([128, 480], F32, tag="pp")
                        nc.tensor.matmul(p, w1a[:, ts(m, 128)], din0f[:, sl],
                                         start=True, stop=False)
                        nc.tensor.matmul(p, w1b[:, ts(m, 128)], din1f[:, sl],
                                         start=False, stop=True)
                        nc.scalar.activation(pre1[:, m, sl], p, AF.Relu)
                for m in range(2):
                    for nch in range(NCH):
                        sl = ts(nch, 480)
                        p = pps.tile([128, 480], F32, tag="pp")
                        for k in range(2):
                            nc.tensor.matmul(p, w2[:, k, ts(m, 128)], pre1[:, k, sl],
                                             start=(k == 0), stop=(k == 1))
                        nc.scalar.activation(pre2[:, m, sl], p, AF.Relu)

                # mem transposed [e%128, ec, b, tc, t%128] then pm = wmT.T @ memT
                for b in range(BL):
                    memT_b = tpool.tile([128, 4, 4, 120], F32, tag="memT")
                    for tc_i in range(4):
                        for ec in range(4):
                            pt = pps.tile([128, 120], F32, tag="pp")
                            nc.tensor.transpose(
                                pt, mem_sb[:, b, tc_i, ts(ec, 128)],
                                ident[0:120, 0:120])
                            nc.vector.tensor_copy(memT_b[:, ec, tc_i, :], pt)
                    pmp = pps.tile([128, T_ENC], F32, tag="pp")
                    for ec in range(4):
                        nc.tensor.matmul(
                            pmp, wm[:, ec, :],
                            memT_b[:, ec, :, :].rearrange("p tc t -> p (tc t)"),
                            start=(ec == 0), stop=(ec == 3))
                    nc.vector.tensor_copy(pm[:, b, :], pmp)

            # ---------- steady-state loop ----------
            with (
                tc.tile_pool(name="wbuf", bufs=6) as wpool,
                tc.tile_pool(name="step", bufs=2) as stp,
                tc.tile_pool(name="im2col", bufs=1) as imp,
                tc.tile_pool(name="srow", bufs=2) as srp,
                tc.tile_pool(name="pstep", bufs=2, space="PSUM") as psp,
            ):
                def lstm_pointwise(gates, c2x, h_out):
                    """gates [128,256] preact (i,f,g,o)x(chunk,b); updates c2x (=2c), h_out."""
                    gi, gf = gates[:, 0:64], gates[:, 64:128]
                    gg, go = gates[:, 128:192], gates[:, 192:256]
                    ti = stp.tile([128, 64], F32, tag="ti")
                    tf = stp.tile([128, 64], F32, tag="tf")
                    tg = stp.tile([128, 64], F32, tag="tg")
                    to = stp.tile([128, 64], F32, tag="to")
                    nc.scalar.activation(ti, gi, AF.Tanh, scale=0.5)
                    nc.scalar.activation(tf, gf, AF.Tanh, scale=0.5)
                    nc.scalar.activation(tg, gg, AF.Tanh)
                    nc.scalar.activation(to, go, AF.Tanh, scale=0.5)
                    m1 = stp.tile([128, 64], F32, tag="m1")
                    nc.vector.tensor_tensor(m1, tf, c2x, OP.mult)      # tf*2c
                    a1 = stp.tile([128, 64], F32, tag="a1")
                    nc.vector.tensor_tensor(a1, m1, c2x, OP.add)       # (1+tf)*2c
                    m2 = stp.tile([128, 64], F32, tag="m2")
                    nc.vector.tensor_tensor(m2, ti, tg, OP.mult)
                    a2 = stp.tile([128, 64], F32, tag="a2")
                    nc.vector.tensor_tensor(a2, m2, tg, OP.add)        # (1+ti)*tg
                    # c2x_new = 2*(sig(f)c + sig(i)tg) = 0.5*a1 + a2
                    nc.vector.scalar_tensor_tensor(
                        out=c2x, in0=a1, scalar=0.5, in1=a2,
                        op0=OP.mult, op1=OP.add)
                    thc = stp.tile([128, 64], F32, tag="thc")
                    nc.scalar.activation(thc, c2x, AF.Tanh, scale=0.5)  # tanh(c)
                    h1 = stp.tile([128, 64], F32, tag="h1")
                    nc.vector.tensor_tensor(h1, to, thc, OP.mult)
                    nc.vector.tensor_tensor(h1, h1, thc, OP.add)
                    nc.vector.tensor_scalar_mul(h_out, h1, 0.5)

                def body(i):
                    # ---- aLSTM gates ----
                    pg = psp.tile([128, 256], F32, tag="gates")
                    for m in range(M_G):
                        wa_m = wpool.tile([128, KA, 128], FP8, tag="wa")
                        eng = (nc.sync, nc.scalar, nc.gpsimd)[m % 3]
                        eng.dma_start(
                            out=wa_m, in_=d_wa.ap()[m].rearrange("k p q -> p k q"))
                        for k in range(KA):
                            if k < 2:
                                rhs = pre2[:, k, ts(i, BL)]
                            elif k < 6:
                                rhs = ctxT[:, k - 2, :]
                            else:
                                rhs = ah[:, k - 6, :]
                            nc.tensor.matmul(
                                pg[:, ts(m, BL)], wa_m[:, k, :], rhs,
                                start=(k == 0), stop=(k == KA - 1))
                    gsb = stp.tile([128, 256], F32, tag="gsb")
                    nc.vector.tensor_tensor(gsb, pg, bias_a, OP.add)
                    lstm_pointwise(gsb, c2a, ah.rearrange("p c b -> p (c b)"))

                    # ---- attention ----
                    pq = psp.tile([128, BL], F32, tag="loc")
                    for k in range(8):
                        nc.tensor.matmul(pq, wq[:, k, :], ah[:, k, :],
                                         start=(k == 0), stop=(k == 7))
                    qT = stp.tile([128, BL], F32, tag="qT")
                    nc.vector.tensor_copy(qT, pq)

                    # im2col: rhs62 [62, b, t], row k reads aw[b, k:k+480]
                    rhs62 = imp.tile([2 * LOCK, BL, T_ENC], F32, tag="rhs62")
                    for kk in range(LOCK):
                        nc.sync.dma_start(
                            out=rhs62[kk:kk + 1, :, :],
                            in_=aw_t[:, kk:kk + T_ENC])
                        nc.sync.dma_start(
                            out=rhs62[LOCK + kk:LOCK + kk + 1, :, :],
                            in_=awc_t[:, kk:kk + T_ENC])

                    e_sb = stp.tile([BL, T_ENC], F32, tag="e_sb")
                    for b in range(BL):
                        ploc_b = psp.tile([128, T_ENC], F32, tag="loc")
                        nc.tensor.matmul(ploc_b, w2t, rhs62[:, b, :],
                                         start=True, stop=True)
                        s1 = srp.tile([128, T_ENC], F32, tag="s1")
                        nc.vector.tensor_tensor(s1, ploc_b, pm[:, b, :], OP.add)
                        s2 = srp.tile([128, T_ENC], F32, tag="s2")
                        nc.scalar.activation(s2, s1, AF.Tanh, bias=qT[:, b:b + 1])
                        e_b = psp.tile([1, 512], F32, tag="row")
                        nc.tensor.matmul(e_b[0:1, 0:T_ENC], v_col, s2,
                                         start=True, stop=True)
                        erow = srp.tile([1, T_ENC], F32, tag="erow")
                        nc.scalar.copy(erow, e_b[0:1, 0:T_ENC])
                        nc.sync.dma_start(out=e_sb[b:b + 1, :], in_=erow)

                    # softmax over t (|e| <= sum|v| so exp is safe without max-sub)
                    esum = stp.tile([BL, 1], F32, tag="esum")
                    expe = stp.tile([BL, T_ENC], F32, tag="expe")
                    nc.scalar.activation(expe, e_sb, AF.Exp, accum_out=esum)
                    rsum = stp.tile([BL, 1], F32, tag="rsum")
                    nc.vector.reciprocal(rsum, esum)
                    nc.vector.tensor_scalar(
                        out=aw_t[:, PAD:PAD + T_ENC], in0=expe,
                        scalar1=rsum, scalar2=None, op0=OP.mult)
                    nc.vector.tensor_tensor(
                        awc_t[:, PAD:PAD + T_ENC],
                        awc_t[:, PAD:PAD + T_ENC],
                        aw_t[:, PAD:PAD + T_ENC], OP.add)
                    nc.sync.dma_start(
                        out=d_align.ap()[ts(i, BL)],
                        in_=aw_t[:, PAD:PAD + T_ENC])

                    # w transposed [t%128, tc, b] then ctx rows -> ctxT scatter
                    w_t = stp.tile([120, 4, BL], F32, tag="w_t")
                    for tc_i in range(4):
                        ptw = psp.tile([120, BL], F32, tag="loc")
                        nc.tensor.transpose(
                            ptw, aw_t[:, PAD + 120 * tc_i:PAD + 120 * (tc_i + 1)],
                            ident[0:BL, 0:BL])
                        nc.vector.tensor_copy(w_t[:, tc_i, :], ptw)
                    ctx_rows = stp.tile([BL, E], F32, tag="ctx_rows")
                    for b in range(BL):
                        ctx_b = psp.tile([1, 512], F32, tag="row")
                        for tc_i in range(4):
                            nc.tensor.matmul(
                                ctx_b, w_t[:, tc_i, b:b + 1],
                                mem_sb[:, b, tc_i, :],
                                start=(tc_i == 0), stop=(tc_i == 3))
                        crow = srp.tile([1, E], F32, tag="crow")
                        nc.scalar.copy(crow, ctx_b)
                        nc.sync.dma_start(out=ctx_rows[b:b + 1, :], in_=crow)
                    for ec in range(4):
                        ptc = psp.tile([128, BL], F32, tag="loc")
                        nc.tensor.transpose(
                            ptc, ctx_rows[:, ts(ec, 128)], ident[0:BL, 0:BL])
                        nc.vector.tensor_copy(ctxT[:, ec, :], ptc)

                    # ---- dLSTM ----
                    pgd = psp.tile([128, 256], F32, tag="gates")
                    for m in range(M_G):
                        wd_m = wpool.tile([128, KD, 128], FP8, tag="wd")
                        eng = (nc.sync, nc.scalar, nc.gpsimd)[m % 3]
                        eng.dma_start(
                            out=wd_m, in_=d_wd.ap()[m].rearrange("k p q -> p k q"))
                        for k in range(KD):
                            if k < 8:
                                rhs = ah[:, k, :]
                            elif k < 12:
                                rhs = ctxT[:, k - 8, :]
                            else:
                                rhs = dh[:, k - 12, :]
                            nc.tensor.matmul(
                                pgd[:, ts(m, BL)], wd_m[:, k, :], rhs,
                                start=(k == 0), stop=(k == KD - 1))
                    gsd = stp.tile([128, 256], F32, tag="gsb")
                    nc.vector.tensor_tensor(gsd, pgd, bias_d, OP.add)
                    lstm_pointwise(gsd, c2d, dh.rearrange("p c b -> p (c b)"))

                    # ---- projection ----
                    po0 = psp.tile([128, BL], F32, tag="loc")
                    for k in range(KP):
                        rhs = dh[:, k, :] if k < 8 else ctxT[:, k - 8, :]
                        nc.tensor.matmul(po0, wp[:, k, 0:128], rhs,
                                         start=(k == 0), stop=(k == KP - 1))
                    so0 = stp.tile([128, BL], F32, tag="so0")
                    nc.vector.tensor_scalar(out=so0, in0=po0, scalar1=pb0,
                                            scalar2=None, op0=OP.add)
                    nc.sync.dma_start(out=d_spec0.ap()[ts(i, 128)], in_=so0)
                    po1 = psp.tile([32, BL], F32, tag="loc")
                    for k in range(KP):
                        rhs = dh[:, k, :] if k < 8 else ctxT[:, k - 8, :]
                        nc.tensor.matmul(po1, wp[:, k, 128:160], rhs,
                                         start=(k == 0), stop=(k == KP - 1))
                    so1 = stp.tile([32, BL], F32, tag="so1")
                    nc.vector.tensor_scalar(out=so1, in0=po1, scalar1=pb1,
                                            scalar2=None, op0=OP.add)
                    nc.sync.dma_start(out=d_spec1.ap()[ts(i, 32)], in_=so1)

                if unroll_python:
                    for i in range(n_steps):
                        body(i)
                else:
                    with tc.For_i(0, n_steps) as i:
                        body(i)

    nc.finalize()
    return nc


def _host_prep(inputs):
    """Build per-core in_maps from full inputs (numpy reshapes only)."""
    mem = np.ascontiguousarray(inputs["memory"], np.float32)
    target = np.ascontiguousarray(inputs["target"], np.float32)

    x = target.transpose(0, 2, 1).reshape(B, STEPS, FRAME)
    inp = np.concatenate([np.zeros((B, 1, FRAME), np.float32), x[:, :STEPS - 1]], 1)
    din = np.ascontiguousarray(inp.transpose(2, 1, 0))  # [160, 300, B]

    wih_a = inputs["arnn_wih"].astype(np.float32)
    whh_a = inputs["arnn_whh"].astype(np.float32)
    wih_d = inputs["drnn_wih"].astype(np.float32)
    whh_d = inputs["drnn_whh"].astype(np.float32)

    wa_t = np.concatenate([wih_a.T, whh_a.T], 0)    # [1792, 4096] rows: pre,ctx,ah
    wd_t = np.concatenate([wih_d.T, whh_d.T], 0)    # [2560, 4096] rows: ah,ctx,dh
    wa_r = np.ascontiguousarray(
        wa_t.reshape(KA, 128, M_G, 128).transpose(2, 0, 1, 3)).astype(
        ml_dtypes.float8_e4m3)
    wd_r = np.ascontiguousarray(
        wd_t.reshape(KD, 128, M_G, 128).transpose(2, 0, 1, 3)).astype(
        ml_dtypes.float8_e4m3)

    wq_t = np.ascontiguousarray(inputs["att_wq"].T.reshape(8, 128, ATTD)).astype(ml_dtypes.bfloat16)
    wm_t = np.ascontiguousarray(inputs["att_wmem"].T.reshape(4, 128, ATTD), np.float32)
    wp_t = np.ascontiguousarray(inputs["proj_w"].T.reshape(KP, 128, FRAME)).astype(ml_dtypes.bfloat16)
    w1_t = np.ascontiguousarray(inputs["prenet_w1"].T, np.float32)
    w2_t = np.ascontiguousarray(inputs["prenet_w2"].T.reshape(2, 128, PRE), np.float32)
    convw = np.ascontiguousarray(inputs["att_loc_conv"].reshape(LOCF, 2 * LOCK), np.float32)
    wl_t = np.ascontiguousarray(inputs["att_wloc"].T, np.float32)
    v_col = np.ascontiguousarray(inputs["att_v"].T, np.float32)

    ba = (inputs["arnn_bih"] + inputs["arnn_bhh"]).astype(np.float32).reshape(M_G, 128)
    bias_a = np.ascontiguousarray(np.repeat(ba.T[:, :, None], BL, 2).reshape(128, 256))
    bd = (inputs["drnn_bih"] + inputs["drnn_bhh"]).astype(np.float32).reshape(M_G, 128)
    bias_d = np.ascontiguousarray(np.repeat(bd.T[:, :, None], BL, 2).reshape(128, 256))
    pb = np.ascontiguousarray(inputs["proj_b"].reshape(FRAME, 1), np.float32)

    shared = dict(
        wa_r=wa_r, wd_r=wd_r, wq_t=wq_t, wm_t=wm_t, wp_t=wp_t,
        w1_t=w1_t, w2_t=w2_t, convw=convw, wloc_t=wl_t, v_col=v_col,
        bias_a=bias_a, bias_d=bias_d, proj_b=pb,
    )
    in_maps = []
    for c in range(NC):
        bs = slice(c * BL, (c + 1) * BL)
        m = dict(shared)
        m["memory_s"] = np.ascontiguousarray(mem[bs])
        m["din0"] = np.ascontiguousarray(din[0:128, :, bs])
        m["din1"] = np.ascontiguousarray(din[128:160, :, bs])
        in_maps.append(m)
    return in_maps


def _host_post(results, n_steps=STEPS):
    specs = np.zeros((STEPS, B, FRAME), np.float32)
    aligns = np.zeros((B, STEPS, T_ENC), np.float32)
    for c, r in enumerate(results):
        bs = slice(c * BL, (c + 1) * BL)
        s0 = r["spec0"].reshape(STEPS, 128, BL)
        s1 = r["spec1"].reshape(STEPS, 32, BL)
        specs[:, bs, 0:128] = s0.transpose(0, 2, 1)
        specs[:, bs, 128:160] = s1.transpose(0, 2, 1)
        aligns[bs] = r["align"].reshape(STEPS, BL, T_ENC).transpose(1, 0, 2)
    spec_out = specs.transpose(1, 0, 2).reshape(B, STEPS * NFPS, N_MELS).transpose(0, 2, 1)
    return spec_out, aligns


_BUILT = {}


def kernel(**inputs):
    n_steps = int(os.environ.get("KSTEPS", STEPS))
    unroll = os.environ.get("KUNROLL", "0") == "1"
    key = (n_steps, unroll)
    if key not in _BUILT:
        _BUILT[key] = build_nc(n_steps, unroll)
    nc = _BUILT[key]
    in_maps = _host_prep(inputs)
    trace = os.environ.get("KTRACE", "0") == "1"
    res = run_bass_kernel_spmd(nc, in_maps, core_ids=list(range(NC)), trace=trace)
    kernel.last_results = res
    return _host_post(res.results, n_steps)


# revision 17
# speedup vs baseline: 1.3367x; 1.0566x over previous
"""Tacotron2-style attention decoder on 8 trn2 NeuronCores.

Sharding: data-parallel over batch (B=64 -> 8 per core, params replicated).
Each core runs the full 300-step recurrence for its 8 batch rows.

Layouts (per core, B_loc=8):
  hidden states transposed: ah/dh [128, 8(chunk), 8(b)], h dim = chunk*128+p
  cell states (2*c actually): c2a/c2d [128, 64] cols (chunk, b)
  ctxT [128, 4(echunk), 8(b)]
  gates psum [128, 256] cols (m, b), m = gate*8+chunk
  attention: pm [128(a), 8(b), 480(t)], per-b s tiles [128, 480]
  aw/awc: awcat [16, 510] rows 0-7 aw, 8-15 awc; 15 zero-pad cols each side

LSTM pointwise uses sigmoid(x) = 0.5*(1+tanh(x/2)) so the steady loop only
needs the exp/tanh ACT table set (one table load total).
"""

import os
import ml_dtypes
import numpy as np
import concourse.bass as bass
import concourse.tile as tile
from concourse import bacc, mybir
from concourse.bass import ts
from concourse.bass_utils import run_bass_kernel_spmd
from concourse.masks import make_identity

F32 = mybir.dt.float32
BF16 = mybir.dt.bfloat16
FP8 = mybir.dt.float8e4
AF = mybir.ActivationFunctionType
OP = mybir.AluOpType

B, T_ENC, E = 64, 480, 512
N_MELS, NFPS, T_MEL = 80, 2, 600
ARNN, DRNN, PRE, ATTD, LOCF, LOCK = 1024, 1024, 256, 128, 32, 31
FRAME = N_MELS * NFPS          # 160
STEPS = T_MEL // NFPS          # 300
PAD = (LOCK - 1) // 2          # 15
NC = 8
BL = B // NC                   # 8 per core
TP = T_ENC + 2 * PAD           # 510 padded time axis

KA = 14   # aLSTM contraction chunks: pre(2) + ctx(4) + ah(8)
KD = 20   # dLSTM: ah(8) + ctx(4) + dh(8)
KP = 12   # proj: dh(8) + ctx(4)
M_G = 32  # gate dim 4096 / 128


def build_nc(n_steps=STEPS, unroll_python=False):
    nc = bacc.Bacc("TRN2", target_bir_lowering=False, debug=False, num_devices=NC)

    # ---- per-core inputs ----
    d_mem = nc.dram_tensor("memory_s", [BL, T_ENC, E], F32, kind="ExternalInput")
    d_din0 = nc.dram_tensor("din0", [128, STEPS, BL], F32, kind="ExternalInput")
    d_din1 = nc.dram_tensor("din1", [32, STEPS, BL], F32, kind="ExternalInput")
    # streamed big weights, rearranged per m-chunk: [M_G, K, 128, 128]
    d_wa = nc.dram_tensor("wa_r", [M_G, 128, KA, 128], FP8, kind="ExternalInput")
    d_wd = nc.dram_tensor("wd_r", [M_G, 128, KD, 128], FP8, kind="ExternalInput")
    # small resident weights
    d_wq = nc.dram_tensor("wq_t", [8, 128, ATTD], BF16, kind="ExternalInput")
    d_wm = nc.dram_tensor("wm_t", [4, 128, ATTD], F32, kind="ExternalInput")
    d_wp = nc.dram_tensor("wp_t", [KP, 128, FRAME], BF16, kind="ExternalInput")
    d_w1 = nc.dram_tensor("w1_t", [FRAME, PRE], F32, kind="ExternalInput")
    d_w2 = nc.dram_tensor("w2_t", [2, 128, PRE], F32, kind="ExternalInput")
    d_cw = nc.dram_tensor("convw", [LOCF, 2 * LOCK], F32, kind="ExternalInput")
    d_wl = nc.dram_tensor("wloc_t", [LOCF, ATTD], F32, kind="ExternalInput")
    d_v = nc.dram_tensor("v_col", [ATTD, 1], F32, kind="ExternalInput")
    d_ba = nc.dram_tensor("bias_a", [128, 256], F32, kind="ExternalInput")
    d_bd = nc.dram_tensor("bias_d", [128, 256], F32, kind="ExternalInput")
    d_pb = nc.dram_tensor("proj_b", [FRAME, 1], F32, kind="ExternalInput")

    # ---- per-core outputs (step-flattened first dim) ----
    d_spec0 = nc.dram_tensor("spec0", [STEPS * 128, BL], F32, kind="ExternalOutput")
    d_spec1 = nc.dram_tensor("spec1", [STEPS * 32, BL], F32, kind="ExternalOutput")
    d_align = nc.dram_tensor("align", [STEPS * BL, T_ENC], F32, kind="ExternalOutput")

    with tile.TileContext(nc) as tc:
        with (
            tc.tile_pool(name="const", bufs=1) as cpool,
            tc.tile_pool(name="state", bufs=1) as spool,
        ):
            # ---------- resident tiles ----------
            wq = cpool.tile([128, 8, ATTD], BF16)
            nc.sync.dma_start(out=wq, in_=d_wq.ap().rearrange("k p a -> p k a"))
            wm = cpool.tile([128, 4, ATTD], F32)
            nc.sync.dma_start(out=wm, in_=d_wm.ap().rearrange("k p a -> p k a"))
            wp = cpool.tile([128, KP, FRAME], BF16)
            nc.sync.dma_start(out=wp, in_=d_wp.ap().rearrange("k p a -> p k a"))
            w1a = cpool.tile([128, PRE], F32)
            nc.sync.dma_start(out=w1a, in_=d_w1.ap()[0:128, :])
            w1b = cpool.tile([32, PRE], F32)
            nc.sync.dma_start(out=w1b, in_=d_w1.ap()[128:160, :])
            w2 = cpool.tile([128, 2, PRE], F32)
            nc.sync.dma_start(out=w2, in_=d_w2.ap().rearrange("k p a -> p k a"))
            cw = cpool.tile([LOCF, 2 * LOCK], F32)
            nc.sync.dma_start(out=cw, in_=d_cw.ap())
            wl = cpool.tile([LOCF, ATTD], F32)
            nc.sync.dma_start(out=wl, in_=d_wl.ap())
            v_col = cpool.tile([ATTD, 1], F32)
            nc.sync.dma_start(out=v_col, in_=d_v.ap())
            bias_a = cpool.tile([128, 256], F32)
            nc.sync.dma_start(out=bias_a, in_=d_ba.ap())
            bias_d = cpool.tile([128, 256], F32)
            nc.sync.dma_start(out=bias_d, in_=d_bd.ap())
            pb0 = cpool.tile([128, 1], F32)
            nc.sync.dma_start(out=pb0, in_=d_pb.ap()[0:128, :])
            pb1 = cpool.tile([32, 1], F32)
            nc.sync.dma_start(out=pb1, in_=d_pb.ap()[128:160, :])

            # memory in [t-part, b, tchunk, e] layout (ctx moving operand)
            # t axis chunked as 4 x 120 (480 = 4*120)
            mem_sb = cpool.tile([120, BL, 4, E], F32)
            nc.sync.dma_start(
                out=mem_sb,
                in_=d_mem.ap().rearrange("b (tc p) e -> p b tc e", p=120),
            )

            ident = cpool.tile([128, 128], F32)
            make_identity(nc, ident)
            pre2 = cpool.tile([128, 2, STEPS * BL], BF16)      # prenet output.T
            pm = cpool.tile([128, BL, T_ENC], F32)             # processed memory.T
            w2t = cpool.tile([2 * LOCK, ATTD], F32)            # fused conv+loc-proj

            # ---------- state tiles ----------
            ah = spool.tile([128, 8, BL], BF16)
            c2a = spool.tile([128, 8 * BL], F32)
            dh = spool.tile([128, 8, BL], BF16)
            c2d = spool.tile([128, 8 * BL], F32)
            ctxT = spool.tile([128, 4, BL], BF16)
            aw_t = spool.tile([BL, TP], F32)
            awc_t = spool.tile([BL, TP], F32)
            for t_ in (ah, c2a, dh, c2d, ctxT, aw_t, awc_t):
                nc.vector.memset(t_, 0.0)

            # ---------- P0: one-time precompute ----------
            with (
                tc.tile_pool(name="p0tmp", bufs=2) as tpool,
                tc.tile_pool(name="p0psum", bufs=2, space="PSUM") as pps,
                tc.tile_pool(name="p0big", bufs=1) as bigpool,
            ):
                din0 = bigpool.tile([128, STEPS, BL], F32)
                nc.sync.dma_start(out=din0, in_=d_din0.ap())
                din1 = bigpool.tile([32, STEPS, BL], F32)
                nc.sync.dma_start(out=din1, in_=d_din1.ap())

                ps_w2 = pps.tile([2 * LOCK, ATTD], F32, tag="pp")
                nc.tensor.matmul(ps_w2, cw, wl, start=True, stop=True)
                nc.vector.tensor_copy(w2t, ps_w2)

                # prenet layer 1: pre1.T [256, 2400] = relu(w1T.T @ din)
                NSTEP = STEPS * BL  # 2400
                NCH = NSTEP // 480  # 5 chunks of 480
                din0f = din0.rearrange("p t b -> p (t b)")
                din1f = din1.rearrange("p t b -> p (t b)")
                pre1 = bigpool.tile([128, 2, NSTEP], F32)
                for m in range(2):
                    for nch in range(NCH):
                        sl = ts(nch, 480)
                        p = pps.tile([128, 480], F32, tag="pp")
                        nc.tensor.matmul(p, w1a[:, ts(m, 128)], din0f[:, sl],
                                         start=True, stop=False)
                        nc.tensor.matmul(p, w1b[:, ts(m, 128)], din1f[:, sl],
                                         start=False, stop=True)
                        nc.scalar.activation(pre1[:, m, sl], p, AF.Relu)
                for m in range(2):
                    for nch in range(NCH):
                        sl = ts(nch, 480)
                        p = pps.tile([128, 480], F32, tag="pp")
                        for k in range(2):
                            nc.tensor.matmul(p, w2[:, k, ts(m, 128)], pre1[:, k, sl],
                                             start=(k == 0), stop=(k == 1))
                        nc.scalar.activation(pre2[:, m, sl], p, AF.Relu)

                # mem transposed [e%128, ec, b, tc, t%128] then pm = wmT.T @ memT
                for b in range(BL):
                    memT_b = tpool.tile([128, 4, 4, 120], F32, tag="memT")
                    for tc_i in range(4):
                        for ec in range(4):
                            pt = pps.tile([128, 120], F32, tag="pp")
                            nc.tensor.transpose(
                                pt, mem_sb[:, b, tc_i, ts(ec, 128)],
                                ident[0:120, 0:120])
                            nc.vector.tensor_copy(memT_b[:, ec, tc_i, :], pt)
                    pmp = pps.tile([128, T_ENC], F32, tag="pp")
                    for ec in range(4):
                        nc.tensor.matmul(
                            pmp, wm[:, ec, :],
                            memT_b[:, ec, :, :].rearrange("p tc t -> p (tc t)"),
                            start=(ec == 0), stop=(ec == 3))
                    nc.vector.tensor_copy(pm[:, b, :], pmp)

            # ---------- steady-state loop ----------
            with (
                tc.tile_pool(name="wbuf", bufs=6) as wpool,
                tc.tile_pool(name="step", bufs=2) as stp,
                tc.tile_pool(name="im2col", bufs=1) as imp,
                tc.tile_pool(name="srow", bufs=2) as srp,
                tc.tile_pool(name="pstep", bufs=2, space="PSUM") as psp,
            ):
                def lstm_pointwise(gates, c2x, h_out):
                    """gates [128,256] preact (i,f,g,o)x(chunk,b); updates c2x (=2c), h_out."""
                    gi, gf = gates[:, 0:64], gates[:, 64:128]
                    gg, go = gates[:, 128:192], gates[:, 192:256]
                    ti = stp.tile([128, 64], F32, tag="ti")
                    tf = stp.tile([128, 64], F32, tag="tf")
                    tg = stp.tile([128, 64], F32, tag="tg")
                    to = stp.tile([128, 64], F32, tag="to")
                    nc.scalar.activation(ti, gi, AF.Tanh, scale=0.5)
                    nc.scalar.activation(tf, gf, AF.Tanh, scale=0.5)
                    nc.scalar.activation(tg, gg, AF.Tanh)
                    nc.scalar.activation(to, go, AF.Tanh, scale=0.5)
                    m1 = stp.tile([128, 64], F32, tag="m1")
                    nc.vector.tensor_tensor(m1, tf, c2x, OP.mult)      # tf*2c
                    a1 = stp.tile([128, 64], F32, tag="a1")
                    nc.vector.tensor_tensor(a1, m1, c2x, OP.add)       # (1+tf)*2c
                    m2 = stp.tile([128, 64], F32, tag="m2")
                    nc.vector.tensor_tensor(m2, ti, tg, OP.mult)
                    a2 = stp.tile([128, 64], F32, tag="a2")
                    nc.vector.tensor_tensor(a2, m2, tg, OP.add)        # (1+ti)*tg
                    # c2x_new = 2*(sig(f)c + sig(i)tg) = 0.5*a1 + a2
                    nc.vector.scalar_tensor_tensor(
                        out=c2x, in0=a1, scalar=0.5, in1=a2,
                        op0=OP.mult, op1=OP.add)
                    thc = stp.tile([128, 64], F32, tag="thc")
                    nc.scalar.activation(thc, c2x, AF.Tanh, scale=0.5)  # tanh(c)
                    h1 = stp.tile([128, 64], F32, tag="h1")
                    nc.vector.tensor_tensor(h1, to, thc, OP.mult)
                    nc.vector.tensor_tensor(h1, h1, thc, OP.add)
                    nc.vector.tensor_scalar_mul(h_out, h1, 0.5)

                def body(i):
                    # ---- aLSTM gates ----
                    pg = psp.tile([128, 256], F32, tag="gates")
                    for m in range(M_G):
                        wa_m = wpool.tile([128, KA, 128], FP8, tag="wa")
                        eng = (nc.sync, nc.scalar, nc.gpsimd)[m % 3]
                        eng.dma_start(out=wa_m, in_=d_wa.ap()[m])
                        for k in range(KA):
                            if k < 2:
                                rhs = pre2[:, k, ts(i, BL)]
                            elif k < 6:
                                rhs = ctxT[:, k - 2, :]
                            else:
                                rhs = ah[:, k - 6, :]
                            nc.tensor.matmul(
                                pg[:, ts(m, BL)], wa_m[:, k, :], rhs,
                                start=(k == 0), stop=(k == KA - 1))
                    gsb = stp.tile([128, 256], F32, tag="gsb")
                    nc.vector.tensor_tensor(gsb, pg, bias_a, OP.add)
                    lstm_pointwise(gsb, c2a, ah.rearrange("p c b -> p (c b)"))

                    # ---- attention ----
                    pq = psp.tile([128, BL], F32, tag="loc")
                    for k in range(8):
                        nc.tensor.matmul(pq, wq[:, k, :], ah[:, k, :],
                                         start=(k == 0), stop=(k == 7))
                    qT = stp.tile([128, BL], F32, tag="qT")
                    nc.vector.tensor_copy(qT, pq)

                    # im2col: rhs62 [62, b, t], row k reads aw[b, k:k+480]
                    rhs62 = imp.tile([2 * LOCK, BL, T_ENC], F32, tag="rhs62")
                    for kk in range(LOCK):
                        nc.sync.dma_start(
                            out=rhs62[kk:kk + 1, :, :],
                            in_=aw_t[:, kk:kk + T_ENC])
                        nc.sync.dma_start(
                            out=rhs62[LOCK + kk:LOCK + kk + 1, :, :],
                            in_=awc_t[:, kk:kk + T_ENC])

                    e_sb = stp.tile([BL, T_ENC], F32, tag="e_sb")
                    for b in range(BL):
                        ploc_b = psp.tile([128, T_ENC], F32, tag="loc")
                        nc.tensor.matmul(ploc_b, w2t, rhs62[:, b, :],
                                         start=True, stop=True)
                        s1 = srp.tile([128, T_ENC], F32, tag="s1")
                        nc.vector.tensor_tensor(s1, ploc_b, pm[:, b, :], OP.add)
                        s2 = srp.tile([128, T_ENC], F32, tag="s2")
                        nc.scalar.activation(s2, s1, AF.Tanh, bias=qT[:, b:b + 1])
                        e_b = psp.tile([1, 512], F32, tag="row")
                        nc.tensor.matmul(e_b[0:1, 0:T_ENC], v_col, s2,
                                         start=True, stop=True)
                        erow = srp.tile([1, T_ENC], F32, tag="erow")
                        nc.scalar.copy(erow, e_b[0:1, 0:T_ENC])
                        nc.sync.dma_start(out=e_sb[b:b + 1, :], in_=erow)

                    # softmax over t (|e| <= sum|v| so exp is safe without max-sub)
                    esum = stp.tile([BL, 1], F32, tag="esum")
                    expe = stp.tile([BL, T_ENC], F32, tag="expe")
                    nc.scalar.activation(expe, e_sb, AF.Exp, accum_out=esum)
                    rsum = stp.tile([BL, 1], F32, tag="rsum")
                    nc.vector.reciprocal(rsum, esum)
                    nc.vector.tensor_scalar(
                        out=aw_t[:, PAD:PAD + T_ENC], in0=expe,
                        scalar1=rsum, scalar2=None, op0=OP.mult)
                    nc.vector.tensor_tensor(
                        awc_t[:, PAD:PAD + T_ENC],
                        awc_t[:, PAD:PAD + T_ENC],
                        aw_t[:, PAD:PAD + T_ENC], OP.add)
                    nc.sync.dma_start(
                        out=d_align.ap()[ts(i, BL)],
                        in_=aw_t[:, PAD:PAD + T_ENC])

                    # w transposed [t%128, tc, b] then ctx rows -> ctxT scatter
                    w_t = stp.tile([120, 4, BL], F32, tag="w_t")
                    for tc_i in range(4):
                        ptw = psp.tile([120, BL], F32, tag="loc")
                        nc.tensor.transpose(
                            ptw, aw_t[:, PAD + 120 * tc_i:PAD + 120 * (tc_i + 1)],
                            ident[0:BL, 0:BL])
                        nc.vector.tensor_copy(w_t[:, tc_i, :], ptw)
                    ctx_rows = stp.tile([BL, E], F32, tag="ctx_rows")
                    for b in range(BL):
                        ctx_b = psp.tile([1, 512], F32, tag="row")
                        for tc_i in range(4):
                            nc.tensor.matmul(
                                ctx_b, w_t[:, tc_i, b:b + 1],
                                mem_sb[:, b, tc_i, :],
                                start=(tc_i == 0), stop=(tc_i == 3))
                        crow = srp.tile([1, E], F32, tag="crow")
                        nc.scalar.copy(crow, ctx_b)
                        nc.sync.dma_start(out=ctx_rows[b:b + 1, :], in_=crow)
                    for ec in range(4):
                        ptc = psp.tile([128, BL], F32, tag="loc")
                        nc.tensor.transpose(
                            ptc, ctx_rows[:, ts(ec, 128)], ident[0:BL, 0:BL])
                        nc.vector.tensor_copy(ctxT[:, ec, :], ptc)

                    # ---- dLSTM ----
                    pgd = psp.tile([128, 256], F32, tag="gates")
                    for m in range(M_G):
                        wd_m = wpool.tile([128, KD, 128], FP8, tag="wd")
                        eng = (nc.sync, nc.scalar, nc.gpsimd)[m % 3]
                        eng.dma_start(out=wd_m, in_=d_wd.ap()[m])
                        for k in range(KD):
                            if k < 8:
                                rhs = ah[:, k, :]
                            elif k < 12:
                                rhs = ctxT[:, k - 8, :]
                            else:
                                rhs = dh[:, k - 12, :]
                            nc.tensor.matmul(
                                pgd[:, ts(m, BL)], wd_m[:, k, :], rhs,
                                start=(k == 0), stop=(k == KD - 1))
                    gsd = stp.tile([128, 256], F32, tag="gsb")
                    nc.vector.tensor_tensor(gsd, pgd, bias_d, OP.add)
                    lstm_pointwise(gsd, c2d, dh.rearrange("p c b -> p (c b)"))

                    # ---- projection ----
                    po0 = psp.tile([128, BL], F32, tag="loc")
                    for k in range(KP):
                        rhs = dh[:, k, :] if k < 8 else ctxT[:, k - 8, :]
                        nc.tensor.matmul(po0, wp[:, k, 0:128], rhs,
                                         start=(k == 0), stop=(k == KP - 1))
                    so0 = stp.tile([128, BL], F32, tag="so0")
                    nc.vector.tensor_scalar(out=so0, in0=po0, scalar1=pb0,
                                            scalar2=None, op0=OP.add)
                    nc.sync.dma_start(out=d_spec0.ap()[ts(i, 128)], in_=so0)
                    po1 = psp.tile([32, BL], F32, tag="loc")
                    for k in range(KP):
                        rhs = dh[:, k, :] if k < 8 else ctxT[:, k - 8, :]
                        nc.tensor.matmul(po1, wp[:, k, 128:160], rhs,
                                         start=(k == 0), stop=(k == KP - 1))
                    so1 = stp.tile([32, BL], F32, tag="so1")
                    nc.vector.tensor_scalar(out=so1, in0=po1, scalar1=pb1,
                                            scalar2=None, op0=OP.add)
                    nc.sync.dma_start(out=d_spec1.ap()[ts(i, 32)], in_=so1)

                if unroll_python:
                    for i in range(n_steps):
                        body(i)
                else:
                    with tc.For_i(0, n_steps) as i:
                        body(i)

    nc.finalize()
    return nc


def _host_prep(inputs):
    """Build per-core in_maps from full inputs (numpy reshapes only)."""
    mem = np.ascontiguousarray(inputs["memory"], np.float32)
    target = np.ascontiguousarray(inputs["target"], np.float32)

    x = target.transpose(0, 2, 1).reshape(B, STEPS, FRAME)
    inp = np.concatenate([np.zeros((B, 1, FRAME), np.float32), x[:, :STEPS - 1]], 1)
    din = np.ascontiguousarray(inp.transpose(2, 1, 0))  # [160, 300, B]

    wih_a = inputs["arnn_wih"].astype(np.float32)
    whh_a = inputs["arnn_whh"].astype(np.float32)
    wih_d = inputs["drnn_wih"].astype(np.float32)
    whh_d = inputs["drnn_whh"].astype(np.float32)

    wa_t = np.concatenate([wih_a.T, whh_a.T], 0)    # [1792, 4096] rows: pre,ctx,ah
    wd_t = np.concatenate([wih_d.T, whh_d.T], 0)    # [2560, 4096] rows: ah,ctx,dh
    wa_r = np.ascontiguousarray(
        wa_t.reshape(KA, 128, M_G, 128).transpose(2, 1, 0, 3)).astype(
        ml_dtypes.float8_e4m3)
    wd_r = np.ascontiguousarray(
        wd_t.reshape(KD, 128, M_G, 128).transpose(2, 1, 0, 3)).astype(
        ml_dtypes.float8_e4m3)

    wq_t = np.ascontiguousarray(inputs["att_wq"].T.reshape(8, 128, ATTD)).astype(ml_dtypes.bfloat16)
    wm_t = np.ascontiguousarray(inputs["att_wmem"].T.reshape(4, 128, ATTD), np.float32)
    wp_t = np.ascontiguousarray(inputs["proj_w"].T.reshape(KP, 128, FRAME)).astype(ml_dtypes.bfloat16)
    w1_t = np.ascontiguousarray(inputs["prenet_w1"].T, np.float32)
    w2_t = np.ascontiguousarray(inputs["prenet_w2"].T.reshape(2, 128, PRE), np.float32)
    convw = np.ascontiguousarray(inputs["att_loc_conv"].reshape(LOCF, 2 * LOCK), np.float32)
    wl_t = np.ascontiguousarray(inputs["att_wloc"].T, np.float32)
    v_col = np.ascontiguousarray(inputs["att_v"].T, np.float32)

    ba = (inputs["arnn_bih"] + inputs["arnn_bhh"]).astype(np.float32).reshape(M_G, 128)
    bias_a = np.ascontiguousarray(np.repeat(ba.T[:, :, None], BL, 2).reshape(128, 256))
    bd = (inputs["drnn_bih"] + inputs["drnn_bhh"]).astype(np.float32).reshape(M_G, 128)
    bias_d = np.ascontiguousarray(np.repeat(bd.T[:, :, None], BL, 2).reshape(128, 256))
    pb = np.ascontiguousarray(inputs["proj_b"].reshape(FRAME, 1), np.float32)

    shared = dict(
        wa_r=wa_r, wd_r=wd_r, wq_t=wq_t, wm_t=wm_t, wp_t=wp_t,
        w1_t=w1_t, w2_t=w2_t, convw=convw, wloc_t=wl_t, v_col=v_col,
        bias_a=bias_a, bias_d=bias_d, proj_b=pb,
    )
    in_maps = []
    for c in range(NC):
        bs = slice(c * BL, (c + 1) * BL)
        m = dict(shared)
        m["memory_s"] = np.ascontiguousarray(mem[bs])
        m["din0"] = np.ascontiguousarray(din[0:128, :, bs])
        m["din1"] = np.ascontiguousarray(din[128:160, :, bs])
        in_maps.append(m)
    return in_maps


def _host_post(results, n_steps=STEPS):
    specs = np.zeros((STEPS, B, FRAME), np.float32)
    aligns = np.zeros((B, STEPS, T_ENC), np.float32)
    for c, r in enumerate(results):
        bs = slice(c * BL, (c + 1) * BL)
        s0 = r["spec0"].reshape(STEPS, 128, BL)
        s1 = r["spec1"].reshape(STEPS, 32, BL)
        specs[:, bs, 0:128] = s0.transpose(0, 2, 1)
        specs[:, bs, 128:160] = s1.transpose(0, 2, 1)
        aligns[bs] = r["align"].reshape(STEPS, BL, T_ENC).transpose(1, 0, 2)
    spec_out = specs.transpose(1, 0, 2).reshape(B, STEPS * NFPS, N_MELS).transpose(0, 2, 1)
    return spec_out, aligns


_BUILT = {}


def kernel(**inputs):
    n_steps = int(os.environ.get("KSTEPS", STEPS))
    unroll = os.environ.get("KUNROLL", "0") == "1"
    key = (n_steps, unroll)
    if key not in _BUILT:
        _BUILT[key] = build_nc(n_steps, unroll)
    nc = _BUILT[key]
    in_maps = _host_prep(inputs)
    trace = os.environ.get("KTRACE", "0") == "1"
    res = run_bass_kernel_spmd(nc, in_maps, core_ids=list(range(NC)), trace=trace)
    kernel.last_results = res
    return _host_post(res.results, n_steps)
